# revision 1
# baseline (speedup 1.0000x reference)
"""GCN-GRU cell fused Trainium2 kernel (8-core data parallel).

Math (per batch b):
    A = d * (adj+I).T * d,  d = rowsum(adj+I)^-0.5
    conc1 = [input, hidden]                (N, 65)
    sig   = sigmoid(A @ conc1 @ W1 + b1)   (N, 128)  node-major flat
    r, u  = first/second half of flat(sig) -> pseudo-node split
    rh    = r * hidden_flat
    c     = tanh(A @ [input, rh] @ W2 + b2)
    out   = u * hidden_flat + (1-u) * c

Implementation notes:
  - batch data-parallel: 8 batches per core, 8 cores.
  - Contraction-side d folded into X on host; output-side d applied on
    PSUM->SBUF copy. adj+I is row-permuted on host into even-rows-then-odd
    order (pi) so the GRU pseudo-node remap becomes plain AP slicing.
  - Big A@X GEMMs run in float32r (full-rate), small W-GEMMs in bf16.
  - A (16.8MB fp32) is streamed from HBM twice; everything else is
    SBUF-resident.
"""

import numpy as np
import ml_dtypes
from contextlib import ExitStack

import concourse.bacc as bacc
import concourse.mybir as mybir
import concourse.tile as tile
from concourse.bass import ts, ds
from concourse.bass_utils import run_bass_kernel_spmd

P = 128
N = 2048
B = 64
H = 64
NCORES = 8
BL = B // NCORES          # 8 batches per core
KT = N // P               # 16 contraction tiles
NT = KT // 2              # 8 (pair-tiles)
CH = N // 512             # 4 output chunks of 512
F32 = mybir.dt.float32
F32R = mybir.dt.float32r
BF16 = mybir.dt.bfloat16
SIG = mybir.ActivationFunctionType.Sigmoid
TANH = mybir.ActivationFunctionType.Tanh

_CACHE = {}


def _build():
    nc = bacc.Bacc("TRN2", target_bir_lowering=False)

    a_d = nc.dram_tensor("a", [N, N], BF16, kind="ExternalInput")
    x1_d = nc.dram_tensor("x1", [N, BL * H], BF16, kind="ExternalInput")
    xin_d = nc.dram_tensor("xin", [P, KT * BL], BF16, kind="ExternalInput")
    hrm_d = nc.dram_tensor("hrm", [BL, N // 2, 2 * H], F32, kind="ExternalInput")
    drep_d = nc.dram_tensor("drep", [P, N], F32, kind="ExternalInput")
    w1h_d = nc.dram_tensor("w1h", [2 * H, 2 * H], BF16, kind="ExternalInput")
    w1i_d = nc.dram_tensor("w1i", [BL + 1, BL, 2 * H], BF16, kind="ExternalInput")
    w2h_d = nc.dram_tensor("w2h", [2 * H, H], BF16, kind="ExternalInput")
    w2i_d = nc.dram_tensor("w2i", [BL + 1, BL, H], BF16, kind="ExternalInput")
    out_d = nc.dram_tensor("out", [BL, N // 2, 2 * H], F32, kind="ExternalOutput")

    out_ap = out_d.ap()

    with tile.TileContext(nc) as tc, ExitStack() as ctx:
        const = ctx.enter_context(tc.tile_pool(name="const", bufs=1))
        x1_sb = const.tile([P, KT, BL * H], BF16)
        xin_sb = const.tile([P, KT, BL], BF16)  # [p, kt, b], host pre-arranged
        hrm_sb = const.tile([P, BL, NT, 2 * H], F32)
        drep_sb = const.tile([P, N], F32)
        w1h_sb = const.tile([2 * H, 2 * H], BF16)
        w1i_sb = const.tile([BL + 1, BL, 2 * H], BF16)
        w2h_sb = const.tile([2 * H, H], BF16)
        w2i_sb = const.tile([BL + 1, BL, H], BF16)
        sig_r = const.tile([P, NT * BL, 2 * H], BF16)   # slot mt*BL+b, mt 0..7
        sig_u = const.tile([P, NT * BL, 2 * H], BF16)   # slot (mt-8)*BL+b
        x2_sb = const.tile([P, KT, BL * H], BF16)
        a_sb = const.tile([P, KT, N], BF16)
        axin_sb = const.tile([BL + 1, N], BF16)         # d*(A@input), row=batch; row 8 = ones (bias row)

        x1_r = x1_d.ap().rearrange("(kt p) f -> p kt f", p=P)
        a_r = a_d.ap().rearrange("(kt p) m -> p kt m", p=P)
        # interleaved fine-grained loads for ch0 so the first matmuls start early
        nc.scalar.dma_start(xin_sb[:], xin_d.ap().rearrange("p (kt b) -> p kt b", b=BL))
        for g in range(4):
            ks = ts(g, 4)
            nc.sync.dma_start(a_sb[:, ks, 0:512], a_r[:, ks, 0:512])
            nc.scalar.dma_start(x1_sb[:, ks, :], x1_r[:, ks, :])
        for ch in range(1, CH):
            nc.sync.dma_start(
                a_sb[:, :, ds(ch * 512, 512)], a_r[:, :, ds(ch * 512, 512)]
            )
        nc.gpsimd.dma_start(hrm_sb[:], hrm_d.ap().rearrange("b (t p) f -> p b t f", p=P))
        nc.gpsimd.dma_start(drep_sb[:], drep_d.ap())
        nc.sync.dma_start(w1h_sb[:], w1h_d.ap())
        nc.vector.memset(axin_sb[:], 1.0)
        nc.sync.dma_start(w1i_sb[:], w1i_d.ap())
        nc.sync.dma_start(w2h_sb[:], w2h_d.ap())
        nc.sync.dma_start(w2i_sb[:], w2i_d.ap())

        axpool = ctx.enter_context(tc.tile_pool(name="ax", bufs=3))
        cpool = ctx.enter_context(tc.tile_pool(name="c", bufs=2))
        gpool = ctx.enter_context(tc.tile_pool(name="g", bufs=3))
        pps = ctx.enter_context(tc.tile_pool(name="ps", bufs=8, space="PSUM"))

        def big_gemm(ch, xsb, with_in):
            n_ps = 5 if with_in else 4
            ps = [
                pps.tile([P, 512], F32, tag="ps", name=f"ps{i}")
                for i in range(n_ps)
            ]
            for kt in range(KT):
                rhs = a_sb[:, kt, ds(ch * 512, 512)]
                st, sp = kt == 0, kt == KT - 1
                for mf in range(4):
                    nc.tensor.matmul(
                        ps[mf][:],
                        lhsT=xsb[:, kt, ts(mf, P)],
                        rhs=rhs, start=st, stop=sp,
                    )
                if with_in:
                    nc.tensor.matmul(
                        ps[4][:BL],
                        lhsT=xin_sb[:, kt, :],
                        rhs=rhs, start=st, stop=sp,
                    )
            axf = axpool.tile([P, 4, 512], BF16, tag="ax")
            for mf in range(4):
                nc.vector.tensor_mul(axf[:, mf, :], ps[mf][:], drep_sb[:, ds(ch * 512, 512)])
            if with_in:
                nc.vector.tensor_mul(
                    axin_sb[:BL, ds(ch * 512, 512)], ps[4][:BL],
                    drep_sb[:BL, ds(ch * 512, 512)],
                )
            return axf

        def emit_w1(ch, axf):
            for mt in range(4 * ch, 4 * ch + 4):
                for b in range(BL):
                    pm = pps.tile([P, 512], F32, tag="ps", name="pm")[:, : 2 * H]
                    nc.tensor.matmul(
                        pm[:],
                        lhsT=axf[64 * (b % 2) : 64 * (b % 2) + 64, b // 2, ts(mt % 4, P)],
                        rhs=w1h_sb[64 * (b % 2) : 64 * (b % 2) + 64, :], start=True, stop=False,
                    )
                    nc.tensor.matmul(
                        pm[:],
                        lhsT=axin_sb[:, ds(mt * P, P)],
                        rhs=w1i_sb[:, b, :], start=False, stop=True,
                    )
                    if mt < NT:
                        dst = sig_r[:, mt * BL + b, :]
                    else:
                        dst = sig_u[:, (mt - NT) * BL + b, :]
                    nc.scalar.activation(dst, pm[:], SIG)

        # ---- GCN1 ----  (W1 for chunk ch-1 emitted after big GEMM of ch, so
        # the PE never stalls on the PSUM->SBUF copies feeding W1's lhsT)
        axfs = {}
        for ch in range(CH):
            axfs[ch] = big_gemm(ch, x1_sb, with_in=True)
            if ch >= 1:
                emit_w1(ch - 1, axfs[ch - 1])
        emit_w1(CH - 1, axfs[CH - 1])

        # ---- X2 assembly: x2[p, kt, (b h)] = sig_r-slice * x1-slice ----
        for kt in range(KT):
            te, jo = (kt, 0) if kt < NT else (kt - NT, 64)
            s3 = sig_r[:, ts(te, BL), jo : jo + 64]
            x13 = x1_sb[:, kt, :].rearrange("p (b h) -> p b h", h=H)
            x23 = x2_sb[:, kt, :].rearrange("p (b h) -> p b h", h=H)
            nc.vector.tensor_mul(x23, s3, x13)

        # ---- GCN2 ----
        def emit_w2_gate(ch, axf2):
            for t in (2 * ch, 2 * ch + 1):
                cs = cpool.tile([P, BL, 2 * H], F32, tag="c")
                for b in range(BL):
                    pc = pps.tile([P, 512], F32, tag="ps", name="pc")[:, : 2 * H]
                    for j in (0, 1):
                        lo = 256 * (t % 2) + j
                        nc.tensor.matmul(
                            pc[:, ds(64 * j, 64)],
                            lhsT=axf2[64 * (b % 2) : 64 * (b % 2) + 64, b // 2, lo : lo + 255 : 2],
                            rhs=w2h_sb[64 * (b % 2) : 64 * (b % 2) + 64, :], start=True, stop=False,
                        )
                        nc.tensor.matmul(
                            pc[:, ds(64 * j, 64)],
                            lhsT=axin_sb[:, 256 * t + j : 256 * t + j + 255 : 2],
                            rhs=w2i_sb[:, b, :], start=False, stop=True,
                        )
                    nc.scalar.activation(cs[:, b, :], pc[:], TANH)
                # gate: out = u*(h - c) + c
                u3 = sig_u[:, ts(t, BL), :]
                h3 = hrm_sb[:, :, t, :]
                g = gpool.tile([P, BL, 2 * H], F32, tag="g")
                nc.vector.tensor_sub(g[:], h3, cs[:])
                nc.vector.tensor_mul(g[:], u3, g[:])
                nc.vector.tensor_add(g[:], g[:], cs[:])
                nc.gpsimd.dma_start(
                    out_ap[:, ts(t, P), :].rearrange("b p f -> p b f"), g[:]
                )

        axf2s = {}
        for ch in range(CH):
            axf2s[ch] = big_gemm(ch, x2_sb, with_in=False)
            if ch >= 1:
                emit_w2_gate(ch - 1, axf2s[ch - 1])
        emit_w2_gate(CH - 1, axf2s[CH - 1])

    nc.finalize()
    return nc


def _prep_inputs(input_tensor, hidden, adj, W1, b1, W2, b2):
    f32 = np.float32
    bf16 = ml_dtypes.bfloat16
    input_tensor = np.ascontiguousarray(input_tensor, f32)
    hidden = np.ascontiguousarray(hidden, f32)
    adj = np.ascontiguousarray(adj, f32)

    pi = np.concatenate([np.arange(0, N, 2), np.arange(1, N, 2)])
    deg = 1.0 + adj.sum(axis=1, dtype=np.float64)
    d = (deg ** -0.5).astype(f32)
    a_perm = np.ascontiguousarray((adj + np.eye(N, dtype=f32))[pi]).astype(bf16)

    drep = np.ascontiguousarray(np.broadcast_to(d, (P, N)), f32)
    w1h = np.ascontiguousarray(np.concatenate([W1[1:], W1[1:]], 0).astype(bf16))
    w1i = np.zeros((BL + 1, BL, 2 * H), bf16)
    for bb in range(BL):
        w1i[bb, bb, :] = W1[0].astype(bf16)
        w1i[BL, bb, :] = b1.astype(bf16)
    w2h = np.ascontiguousarray(np.concatenate([W2[1:], W2[1:]], 0).astype(bf16))
    w2i = np.zeros((BL + 1, BL, H), bf16)
    for bb in range(BL):
        w2i[bb, bb, :] = W2[0].astype(bf16)
        w2i[BL, bb, :] = b2.astype(bf16)

    dh = d[None, :, None] * hidden          # (B, N, H)
    din = d[None, :] * input_tensor         # (B, N)

    in_maps = []
    for c in range(NCORES):
        bs = slice(BL * c, BL * c + BL)
        x1 = np.ascontiguousarray(
            dh[bs][:, pi, :].transpose(1, 0, 2).reshape(N, BL * H)
        ).astype(bf16)
        xin = np.ascontiguousarray(
            din[bs][:, pi].T.reshape(KT, P, BL).transpose(1, 0, 2).reshape(P, KT * BL)
        ).astype(bf16)
        hrm = np.ascontiguousarray(hidden[bs].reshape(BL, N // 2, 2 * H))
        in_maps.append({
            "a": a_perm, "x1": x1, "xin": xin, "hrm": hrm, "drep": drep,
            "w1h": w1h, "w1i": w1i, "w2h": w2h, "w2i": w2i,
        })
    return in_maps


LAST_RESULTS = None


def kernel(input_tensor, hidden, adj, W1, b1, W2, b2):
    global LAST_RESULTS
    if "nc" not in _CACHE:
        _CACHE["nc"] = _build()
    nc = _CACHE["nc"]
    in_maps = _prep_inputs(input_tensor, hidden, adj, W1, b1, W2, b2)
    res = run_bass_kernel_spmd(nc, in_maps, core_ids=list(range(NCORES)))
    LAST_RESULTS = res
    outs = [r["out"] for r in res.results]
    return np.concatenate(outs, axis=0).reshape(B, N, H).astype(np.float32)


if __name__ == "__main__":
    rng = np.random.default_rng(0)
    inputs = {
        "input_tensor": rng.standard_normal((B, N), dtype=np.float32),
        "hidden": rng.standard_normal((B, N, H), dtype=np.float32),
        "adj": rng.random((N, N), dtype=np.float32),
        "W1": rng.standard_normal((H + 1, 2 * H), dtype=np.float32) * 0.15,
        "b1": np.full((2 * H,), 0.4, np.float32),
        "W2": rng.standard_normal((H + 1, H), dtype=np.float32) * 0.15,
        "b2": np.full((H,), 0.6, np.float32),
    }
    out = kernel(**inputs)
    print(out.shape, out.dtype)



# revision 2
# speedup vs baseline: 1.7226x; 1.7226x over previous
"""GCN-GRU cell fused Trainium2 kernel (8-core data parallel), v2.

Math (per batch b):
    A = d * (adj+I).T * d,  d = rowsum(adj+I)^-0.5
    conc1 = [input, hidden]                (N, 65)
    sig   = sigmoid(A @ conc1 @ W1 + b1)   (N, 128)  node-major flat
    r, u  = first/second half of flat(sig) -> pseudo-node split
    rh    = r * hidden_flat
    c     = tanh(A @ [input, rh] @ W2 + b2)
    out   = u * hidden_flat + (1-u) * c

Key implementation choices (vs v1 baseline at 275us):
  - Big A@X GEMMs in fp8e4 with perf_mode=DoubleRow: rhs [128, 2kt, 512]
    streams 1024 fp8 cols -> 512-col f32 PSUM out, ~2x ALU rate. X is
    host-scaled by 32*d so fp8 values sit in e4m3's sweet spot; the 1/32
    and output-side d are folded into the PSUM->SBUF copy (drep).
  - lhsT (the X tile) is reused across 4 output chunks per load: psum
    tag "bg" holds 4 concurrently-accumulating banks (one per chunk).
  - A@input is computed on HOST (tiny: 0.5 GFLOP sgemm) and DMA'd into
    dedicated partition rows of the ax work tile, so the W-stage is a
    SINGLE K~66 matmul per (128-node-group, batch): rows 0-63 ax-feats,
    64/65 (even slots) or 62/63 (odd slots) = axin/ones; unused rows
    zeroed once. Bias rides the ones-row.
  - W-stage psum: one accumulation group per 2KB bank (4 batches share a
    bank; start on first, stop on last) so sigmoid/tanh run as few large
    activations instead of 192 small ones.
  - Gate stage identical to v1 (even/odd node interleave via stride-2
    lhsT), hidden kept f32, output f32.
"""

import numpy as np
import ml_dtypes
from contextlib import ExitStack

import concourse.bacc as bacc
import concourse.mybir as mybir
import concourse.tile as tile
from concourse.bass import ts, ds
from concourse.bass_utils import run_bass_kernel_spmd

P = 128
N = 2048
B = 64
H = 64
NCORES = 8
BL = B // NCORES          # 8 batches per core
KT = N // P               # 16 contraction tiles
NT = KT // 2              # 8 (pair-tiles / half-node groups)
CH = N // 512             # 4 output chunks of 512
F32 = mybir.dt.float32
BF16 = mybir.dt.bfloat16
FP8 = mybir.dt.float8e4
SIG = mybir.ActivationFunctionType.Sigmoid
TANH = mybir.ActivationFunctionType.Tanh
DR = mybir.MatmulPerfMode.DoubleRow

_CACHE = {}


def _build():
    nc = bacc.Bacc("TRN2", target_bir_lowering=False)

    a_d = nc.dram_tensor("a", [N, N], FP8, kind="ExternalInput")
    x1_d = nc.dram_tensor("x1", [N, BL * H], FP8, kind="ExternalInput")
    hrm_d = nc.dram_tensor("hrm", [BL, N // 2, 2 * H], F32, kind="ExternalInput")
    drep_d = nc.dram_tensor("drep", [P, N], F32, kind="ExternalInput")
    axe_d = nc.dram_tensor("axe", [2, BL // 2, N], BF16, kind="ExternalInput")
    axo_d = nc.dram_tensor("axo", [2, BL // 2, N], BF16, kind="ExternalInput")
    w1e_d = nc.dram_tensor("w1e", [P, 2 * H], BF16, kind="ExternalInput")
    w1o_d = nc.dram_tensor("w1o", [P, 2 * H], BF16, kind="ExternalInput")
    w2e_d = nc.dram_tensor("w2e", [P, H], BF16, kind="ExternalInput")
    w2o_d = nc.dram_tensor("w2o", [P, H], BF16, kind="ExternalInput")
    out_d = nc.dram_tensor("out", [BL, N // 2, 2 * H], F32, kind="ExternalOutput")

    out_ap = out_d.ap()

    with tile.TileContext(nc) as tc, ExitStack() as ctx:
        const = ctx.enter_context(tc.tile_pool(name="const", bufs=1))
        a_sb = const.tile([P, KT, N], FP8)
        x1_sb = const.tile([P, KT, BL * H], FP8)
        x2_sb = const.tile([P, KT, BL * H], FP8)
        axw = const.tile([P, BL, N], BF16)
        hrm_sb = const.tile([P, BL, NT, 2 * H], F32)
        drep_sb = const.tile([P, N], F32)
        sig_r = const.tile([P, NT * BL, 2 * H], BF16)
        sig_u = const.tile([P, NT * BL, 2 * H], BF16)
        w1e_sb = const.tile([P, 2 * H], BF16)
        w1o_sb = const.tile([P, 2 * H], BF16)
        w2e_sb = const.tile([P, H], BF16)
        w2o_sb = const.tile([P, H], BF16)

        # zero the ax work tile once: W matmuls contract K=128 with zero
        # rhs rows, so unused lhsT rows must be 0 (not garbage/NaN).
        nc.vector.memset(axw[:], 0.0)

        a_r = a_d.ap().rearrange("(kt p) m -> p kt m", p=P)
        x1_r = x1_d.ap().rearrange("(kt p) f -> p kt f", p=P)
        # a by kt blocks, first-needed-first, split across two queues
        nc.sync.dma_start(a_sb[:, 0:2, :], a_r[:, 0:2, :])
        nc.scalar.dma_start(x1_sb[:], x1_r[:])
        nc.sync.dma_start(a_sb[:, 2:5, :], a_r[:, 2:5, :])
        nc.scalar.dma_start(a_sb[:, 10:13, :], a_r[:, 10:13, :])
        nc.sync.dma_start(a_sb[:, 5:10, :], a_r[:, 5:10, :])
        nc.scalar.dma_start(a_sb[:, 13:16, :], a_r[:, 13:16, :])
        nc.gpsimd.dma_start(drep_sb[:], drep_d.ap())
        nc.gpsimd.dma_start(axw[64:66, 0:BL:2, :], axe_d.ap())
        nc.gpsimd.dma_start(axw[62:64, 1:BL:2, :], axo_d.ap())
        nc.gpsimd.dma_start(w1e_sb[:], w1e_d.ap())
        nc.gpsimd.dma_start(w1o_sb[:], w1o_d.ap())
        nc.gpsimd.dma_start(w2e_sb[:], w2e_d.ap())
        nc.gpsimd.dma_start(w2o_sb[:], w2o_d.ap())
        nc.gpsimd.dma_start(hrm_sb[:], hrm_d.ap().rearrange("b (t p) f -> p b t f", p=P))

        pps = ctx.enter_context(tc.tile_pool(name="ps", bufs=1, space="PSUM"))
        cpool = ctx.enter_context(tc.tile_pool(name="c", bufs=2))
        gpool = ctx.enter_context(tc.tile_pool(name="g", bufs=2))

        def big_gcn(xsb, tagpfx):
            # 4 passes (one per 128-col lhsT slice = batch pair); each
            # lhsT load streams all 4 output chunks (4 psum banks open).
            for mf in range(4):
                ps = [
                    pps.tile([P, 512], F32, tag="bg", bufs=4, name=f"{tagpfx}{mf}c{ch}")
                    for ch in range(CH)
                ]
                for tp in range(NT):
                    lhsT = xsb[:, 2 * tp : 2 * tp + 2, ts(mf, P)]
                    for ch in range(CH):
                        nc.tensor.matmul(
                            ps[ch][:],
                            lhsT=lhsT,
                            rhs=a_sb[:, 2 * tp : 2 * tp + 2, ts(ch, 512)],
                            start=(tp == 0), stop=(tp == NT - 1),
                            perf_mode=DR,
                        )
                # psum (=32*ax_pre) -> axw feats rows, scaled by d/32
                for ch in range(CH):
                    cols = ds(ch * 512, 512)
                    nc.vector.tensor_mul(
                        axw[0:64, 2 * mf, cols], ps[ch][0:64, :], drep_sb[0:64, cols]
                    )
                    nc.vector.tensor_mul(
                        axw[64:128, 2 * mf + 1, cols], ps[ch][64:128, :],
                        drep_sb[64:128, cols],
                    )

        # ---- GCN1 ----
        big_gcn(x1_sb, "p1m")

        # ---- W1 + sigmoid; x2 assembly trails each sig_r group ----
        for mt in range(KT):
            pm = pps.tile([P, BL, 2 * H], F32, tag="pm", bufs=2, name=f"pm{mt}")
            for b in range(BL):
                rhs = w1e_sb if b % 2 == 0 else w1o_sb
                nc.tensor.matmul(
                    pm[:, b, :],
                    lhsT=axw[:, b, ts(mt, P)],
                    rhs=rhs[:],
                    start=(b % 4 == 0), stop=(b % 4 == 3),
                )
            if mt < NT:
                dst = sig_r[:, ts(mt, BL), :]
            else:
                dst = sig_u[:, ts(mt - NT, BL), :]
            nc.scalar.activation(dst, pm[:], SIG)
            if mt < NT:
                for kt in (mt, mt + NT):
                    jo = 0 if kt < NT else 64
                    s3 = sig_r[:, ts(mt, BL), jo : jo + 64]
                    x13 = x1_sb[:, kt, :].rearrange("p (b h) -> p b h", h=H)
                    x23 = x2_sb[:, kt, :].rearrange("p (b h) -> p b h", h=H)
                    nc.vector.tensor_mul(x23, s3, x13)

        # ---- GCN2 ----
        big_gcn(x2_sb, "p2m")

        # ---- W2 + tanh + gate + store ----
        for t in range(NT):
            pcg = pps.tile([P, BL, 2 * H], F32, tag="pm", bufs=2, name=f"pc{t}")
            ch = t // 2
            for b in range(BL):
                rhs = w2e_sb if b % 2 == 0 else w2o_sb
                for j in (0, 1):
                    lo = 512 * ch + 256 * (t % 2) + j
                    nc.tensor.matmul(
                        pcg[:, b, ds(64 * j, 64)],
                        lhsT=axw[:, b, lo : lo + 255 : 2],
                        rhs=rhs[:],
                        start=(b % 4 == 0 and j == 0), stop=(b % 4 == 3 and j == 1),
                    )
            cs = cpool.tile([P, BL, 2 * H], BF16, tag="c")
            nc.scalar.activation(cs[:], pcg[:], TANH)
            # gate: out = u*(h - c) + c
            u3 = sig_u[:, ts(t, BL), :]
            h3 = hrm_sb[:, :, t, :]
            g = gpool.tile([P, BL, 2 * H], F32, tag="g")
            nc.vector.tensor_sub(g[:], h3, cs[:])
            nc.vector.tensor_mul(g[:], u3, g[:])
            nc.vector.tensor_add(g[:], g[:], cs[:])
            nc.gpsimd.dma_start(
                out_ap[:, ts(t, P), :].rearrange("b p f -> p b f"), g[:]
            )

    nc.finalize()
    return nc


def _prep_inputs(input_tensor, hidden, adj, W1, b1, W2, b2):
    f32 = np.float32
    bf16 = ml_dtypes.bfloat16
    fp8 = ml_dtypes.float8_e4m3
    input_tensor = np.ascontiguousarray(input_tensor, f32)
    hidden = np.ascontiguousarray(hidden, f32)
    adj = np.ascontiguousarray(adj, f32)
    W1 = np.asarray(W1, f32); b1 = np.asarray(b1, f32)
    W2 = np.asarray(W2, f32); b2 = np.asarray(b2, f32)

    pi = np.concatenate([np.arange(0, N, 2), np.arange(1, N, 2)])
    a_hat = adj + np.eye(N, dtype=f32)
    deg = a_hat.sum(axis=1, dtype=np.float64)
    d = (deg ** -0.5).astype(f32)
    a_perm = np.ascontiguousarray(a_hat[pi]).astype(fp8)

    drep = np.ascontiguousarray(np.broadcast_to(d / 32.0, (P, N)), f32)

    # host A@input: axin_s[b, m] = d[m] * sum_n a_hat[n, m] * d[n] * in[b, n]
    din = (d[None, :] * input_tensor).astype(f32)
    axin_s = ((din @ a_hat) * d[None, :]).astype(bf16)          # (B, N)

    w1e = np.zeros((P, 2 * H), bf16)
    w1e[0:64] = W1[1:].astype(bf16); w1e[64] = W1[0].astype(bf16)
    w1e[65] = b1.astype(bf16)
    w1o = np.zeros((P, 2 * H), bf16)
    w1o[62] = W1[0].astype(bf16); w1o[63] = b1.astype(bf16)
    w1o[64:128] = W1[1:].astype(bf16)
    w2e = np.zeros((P, H), bf16)
    w2e[0:64] = W2[1:].astype(bf16); w2e[64] = W2[0].astype(bf16)
    w2e[65] = b2.astype(bf16)
    w2o = np.zeros((P, H), bf16)
    w2o[62] = W2[0].astype(bf16); w2o[63] = b2.astype(bf16)
    w2o[64:128] = W2[1:].astype(bf16)

    dh = (32.0 * d[None, :, None] * hidden).astype(f32)         # (B, N, H)

    in_maps = []
    for c in range(NCORES):
        bs = slice(BL * c, BL * c + BL)
        x1 = np.ascontiguousarray(
            dh[bs][:, pi, :].transpose(1, 0, 2).reshape(N, BL * H)
        ).astype(fp8)
        hrm = np.ascontiguousarray(hidden[bs].reshape(BL, N // 2, 2 * H))
        axc = axin_s[bs]                                        # (8, N) bf16
        axe = np.empty((2, BL // 2, N), bf16)
        axe[0] = axc[0:BL:2]; axe[1] = 1.0
        axo = np.empty((2, BL // 2, N), bf16)
        axo[0] = axc[1:BL:2]; axo[1] = 1.0
        in_maps.append({
            "a": a_perm, "x1": x1, "hrm": hrm, "drep": drep,
            "axe": axe, "axo": axo,
            "w1e": w1e, "w1o": w1o, "w2e": w2e, "w2o": w2o,
        })
    return in_maps


LAST_RESULTS = None


def kernel(input_tensor, hidden, adj, W1, b1, W2, b2):
    global LAST_RESULTS
    if "nc" not in _CACHE:
        _CACHE["nc"] = _build()
    nc = _CACHE["nc"]
    in_maps = _prep_inputs(input_tensor, hidden, adj, W1, b1, W2, b2)
    res = run_bass_kernel_spmd(nc, in_maps, core_ids=list(range(NCORES)))
    LAST_RESULTS = res
    outs = [r["out"] for r in res.results]
    return np.concatenate(outs, axis=0).reshape(B, N, H).astype(np.float32)


if __name__ == "__main__":
    rng = np.random.default_rng(0)
    inputs = {
        "input_tensor": rng.standard_normal((B, N), dtype=np.float32),
        "hidden": rng.standard_normal((B, N, H), dtype=np.float32),
        "adj": rng.random((N, N), dtype=np.float32),
        "W1": rng.standard_normal((H + 1, 2 * H), dtype=np.float32) * 0.15,
        "b1": np.full((2 * H,), 0.4, np.float32),
        "W2": rng.standard_normal((H + 1, H), dtype=np.float32) * 0.15,
        "b2": np.full((H,), 0.6, np.float32),
    }
    out = kernel(**inputs)
    print(out.shape, out.dtype)


# revision 3
# speedup vs baseline: 1.9638x; 1.1400x over previous
"""GCN-GRU cell fused Trainium2 kernel (8-core data parallel), v3.

Math (per batch b):
    A = d * (adj+I).T * d,  d = rowsum(adj+I)^-0.5
    conc1 = [input, hidden]                (N, 65)
    sig   = sigmoid(A @ conc1 @ W1 + b1)   (N, 128)  node-major flat
    r, u  = first/second half of flat(sig) -> pseudo-node split
    rh    = r * hidden_flat
    c     = tanh(A @ [input, rh] @ W2 + b2)
    out   = u * hidden_flat + (1-u) * c

Implementation highlights:
  - Big A@X GEMMs in fp8e4 DoubleRow, 512-col f32 PSUM out per matmul,
    lhsT reused across 4 chunk accumulators per load.
  - Both d factors folded on host: A columns pre-scaled by 32*d[m], X
    rows by 32*d[n] (d ~ 2^-5, so ~lossless in fp8); the 2^10 factor is
    divided out of the tiny W matrices host-side. PSUM->SBUF copies are
    then plain dtype casts: GCN1's run on DVE, GCN2's on the scalar
    engine (activation-Copy) to balance engine load.
  - A@input rows computed on host and DMA'd into dedicated partition
    rows of the ax work tile; the W stage is a single K=128 matmul per
    (128-node group, batch) with bias riding a host ones-row.
  - All big inputs are host-prearranged partition-major so DMAs move
    multi-KB contiguous runs per partition; first-needed slices are
    separate DMAs so the PE starts after ~1MB, not ~8MB.
  - W2/gate/store phase split into batch halves: half 0 is emitted
    between GCN2 passes 1 and 2, so the tail only carries half 1.
"""

import numpy as np
import ml_dtypes
from contextlib import ExitStack

import concourse.bacc as bacc
import concourse.mybir as mybir
import concourse.tile as tile
from concourse.bass import ts, ds
from concourse.bass_utils import run_bass_kernel_spmd

P = 128
N = 2048
B = 64
H = 64
NCORES = 8
BL = B // NCORES          # 8 batches per core
KT = N // P               # 16 contraction tiles
NT = KT // 2              # 8 (pair-tiles / half-node groups)
CH = N // 512             # 4 output chunks of 512
F32 = mybir.dt.float32
BF16 = mybir.dt.bfloat16
FP8 = mybir.dt.float8e4
SIG = mybir.ActivationFunctionType.Sigmoid
TANH = mybir.ActivationFunctionType.Tanh
DR = mybir.MatmulPerfMode.DoubleRow

_CACHE = {}


def _build():
    nc = bacc.Bacc("TRN2", target_bir_lowering=False)

    a_d = nc.dram_tensor("a", [P, KT * N], FP8, kind="ExternalInput")
    x1_d = nc.dram_tensor("x1", [P, KT * BL * H], FP8, kind="ExternalInput")
    hrm_d = nc.dram_tensor("hrm", [P, BL, NT, 2 * H], BF16, kind="ExternalInput")
    axe_d = nc.dram_tensor("axe", [64, BL // 2, N], BF16, kind="ExternalInput")
    axo_d = nc.dram_tensor("axo", [64, BL // 2, N], BF16, kind="ExternalInput")
    w1e_d = nc.dram_tensor("w1e", [P, 2 * H], BF16, kind="ExternalInput")
    w1o_d = nc.dram_tensor("w1o", [P, 2 * H], BF16, kind="ExternalInput")
    w2e_d = nc.dram_tensor("w2e", [P, H], BF16, kind="ExternalInput")
    w2o_d = nc.dram_tensor("w2o", [P, H], BF16, kind="ExternalInput")
    out_d = nc.dram_tensor("out", [BL, N // 2, 2 * H], BF16, kind="ExternalOutput")

    out_ap = out_d.ap()

    with tile.TileContext(nc) as tc, ExitStack() as ctx:
        const = ctx.enter_context(tc.tile_pool(name="const", bufs=1))
        a_sb = const.tile([P, KT, N], FP8)
        x1_sb = const.tile([P, KT, BL * H], FP8)
        x2_sb = const.tile([P, KT, BL * H], FP8)
        axw = const.tile([P, BL, N], BF16)
        hrm_sb = const.tile([P, BL, NT, 2 * H], BF16)
        sig_r = const.tile([P, NT * BL, 2 * H], BF16)
        sig_u = const.tile([P, NT * BL, 2 * H], BF16)
        w1e_sb = const.tile([P, 2 * H], BF16)
        w1o_sb = const.tile([P, 2 * H], BF16)
        w2e_sb = const.tile([P, H], BF16)
        w2o_sb = const.tile([P, H], BF16)

        a_r = a_d.ap().rearrange("p (kt m) -> p kt m", m=N)
        x1_r = x1_d.ap().rearrange("p (kt f) -> p kt f", f=BL * H)
        # first-needed-first, split so the PE can start after ~1MB
        nc.sync.dma_start(a_sb[:, 0:2, :], a_r[:, 0:2, :])
        nc.scalar.dma_start(x1_sb[:, 0:8, :], x1_r[:, 0:8, :])
        nc.sync.dma_start(a_sb[:, 2:4, :], a_r[:, 2:4, :])
        nc.scalar.dma_start(x1_sb[:, 8:16, :], x1_r[:, 8:16, :])
        nc.sync.dma_start(a_sb[:, 4:8, :], a_r[:, 4:8, :])
        nc.scalar.dma_start(a_sb[:, 12:16, :], a_r[:, 12:16, :])
        nc.sync.dma_start(a_sb[:, 8:12, :], a_r[:, 8:12, :])
        nc.gpsimd.dma_start(w1e_sb[:], w1e_d.ap())
        nc.gpsimd.dma_start(w1o_sb[:], w1o_d.ap())
        nc.gpsimd.dma_start(w2e_sb[:], w2e_d.ap())
        nc.gpsimd.dma_start(w2o_sb[:], w2o_d.ap())
        nc.gpsimd.dma_start(axw[64:128, 0:BL:2, :], axe_d.ap())
        nc.gpsimd.dma_start(axw[0:64, 1:BL:2, :], axo_d.ap())
        nc.gpsimd.dma_start(hrm_sb[:], hrm_d.ap())

        pps = ctx.enter_context(tc.tile_pool(name="ps", bufs=1, space="PSUM"))
        cpool = ctx.enter_context(tc.tile_pool(name="c", bufs=2))
        gpool = ctx.enter_context(tc.tile_pool(name="g", bufs=2))

        def big_pass(xsb, mf, tagpfx, copy_eng):
            ps = [
                pps.tile([P, 512], F32, tag="bg", bufs=4, name=f"{tagpfx}{mf}c{ch}")
                for ch in range(CH)
            ]
            for tp in range(NT):
                lhsT = xsb[:, 2 * tp : 2 * tp + 2, ts(mf, P)]
                for ch in range(CH):
                    nc.tensor.matmul(
                        ps[ch][:],
                        lhsT=lhsT,
                        rhs=a_sb[:, 2 * tp : 2 * tp + 2, ts(ch, 512)],
                        start=(tp == 0), stop=(tp == NT - 1),
                        perf_mode=DR,
                    )
            # psum (= 2^10 * true ax) -> axw feats rows, plain dtype cast
            for ch in range(CH):
                cols = ds(ch * 512, 512)
                if copy_eng == "v":
                    nc.vector.tensor_copy(out=axw[0:64, 2 * mf, cols], in_=ps[ch][0:64, :])
                    nc.vector.tensor_copy(
                        out=axw[64:128, 2 * mf + 1, cols], in_=ps[ch][64:128, :]
                    )
                else:
                    nc.scalar.copy(axw[0:64, 2 * mf, cols], ps[ch][0:64, :])
                    nc.scalar.copy(axw[64:128, 2 * mf + 1, cols], ps[ch][64:128, :])

        def w2_gate_half(t, hb):
            # hb = 0 or 4: batch half
            pcg = pps.tile([P, 4, 2 * H], F32, tag="pm", bufs=4, name=f"pc{t}_{hb}")
            ch = t // 2
            for i in range(4):
                b = hb + i
                rhs = w2e_sb if b % 2 == 0 else w2o_sb
                for j in (0, 1):
                    lo = 512 * ch + 256 * (t % 2) + j
                    nc.tensor.matmul(
                        pcg[:, i, ds(64 * j, 64)],
                        lhsT=axw[:, b, lo : lo + 255 : 2],
                        rhs=rhs[:],
                        start=(i == 0 and j == 0), stop=(i == 3 and j == 1),
                    )
            cs = cpool.tile([P, 4, 2 * H], BF16, tag="c")
            nc.scalar.activation(cs[:], pcg[:], TANH)
            u3 = sig_u[:, ds(t * BL + hb, 4), :]
            h3 = hrm_sb[:, hb : hb + 4, t, :]
            g = gpool.tile([P, 4, 2 * H], BF16, tag="g")
            nc.vector.tensor_sub(g[:], h3, cs[:])
            nc.vector.tensor_mul(g[:], u3, g[:])
            nc.vector.tensor_add(g[:], g[:], cs[:])
            eng = nc.gpsimd if (t + hb) % 2 == 0 else nc.sync
            eng.dma_start(
                out_ap[hb : hb + 4, ts(t, P), :].rearrange("b p f -> p b f"), g[:]
            )

        # ---- GCN1: 4 reuse-passes, copies on DVE ----
        for mf in range(4):
            big_pass(x1_sb, mf, "p1m", "v")

        # ---- W1 + sigmoid (scalar); x2 assembly (DVE) trails ----
        for mt in range(KT):
            for h in (0, 1):
                pm = pps.tile([P, 4, 2 * H], F32, tag="pm", bufs=4, name=f"pm{mt}_{h}")
                for i in range(4):
                    b = 4 * h + i
                    rhs = w1e_sb if b % 2 == 0 else w1o_sb
                    nc.tensor.matmul(
                        pm[:, i, :],
                        lhsT=axw[:, b, ts(mt, P)],
                        rhs=rhs[:],
                        start=(i == 0), stop=(i == 3),
                    )
                if mt < NT:
                    dst = sig_r[:, ds(mt * BL + 4 * h, 4), :]
                else:
                    dst = sig_u[:, ds((mt - NT) * BL + 4 * h, 4), :]
                nc.scalar.activation(dst, pm[:], SIG)
            if mt < NT:
                for kt in (mt, mt + NT):
                    jo = 0 if kt < NT else 64
                    s3 = sig_r[:, ts(mt, BL), jo : jo + 64]
                    x13 = x1_sb[:, kt, :].rearrange("p (b h) -> p b h", h=H)
                    x23 = x2_sb[:, kt, :].rearrange("p (b h) -> p b h", h=H)
                    nc.vector.tensor_mul(x23, s3, x13)

        # ---- GCN2: copies on scalar; W2 half 0 between passes 1 and 2 ----
        big_pass(x2_sb, 0, "p2m", "s")
        big_pass(x2_sb, 1, "p2m", "s")
        for t in range(NT):
            w2_gate_half(t, 0)
        big_pass(x2_sb, 2, "p2m", "s")
        big_pass(x2_sb, 3, "p2m", "s")
        for t in range(NT):
            w2_gate_half(t, 4)

    nc.finalize()
    return nc


def _prep_inputs(input_tensor, hidden, adj, W1, b1, W2, b2):
    f32 = np.float32
    bf16 = ml_dtypes.bfloat16
    fp8 = ml_dtypes.float8_e4m3
    input_tensor = np.ascontiguousarray(input_tensor, f32)
    hidden = np.ascontiguousarray(hidden, f32)
    adj = np.ascontiguousarray(adj, f32)
    W1 = np.asarray(W1, f32); b1 = np.asarray(b1, f32)
    W2 = np.asarray(W2, f32); b2 = np.asarray(b2, f32)

    pi = np.concatenate([np.arange(0, N, 2), np.arange(1, N, 2)])
    a_hat = adj + np.eye(N, dtype=f32)
    deg = a_hat.sum(axis=1, dtype=np.float64)
    d = (deg ** -0.5).astype(f32)

    # A columns scaled by 32*d[m], rows permuted; partition-major layout
    a_s = (a_hat[pi] * (32.0 * d)[None, :]).astype(fp8)
    a_pre = np.ascontiguousarray(
        a_s.reshape(KT, P, N).transpose(1, 0, 2).reshape(P, KT * N)
    )

    # host A@input: axin[b, m] = 2^10 * d[m] * sum_n a_hat[n,m] d[n] in[b,n]
    din = (d[None, :] * input_tensor).astype(f32)
    axin_s = (1024.0 * (din @ a_hat) * d[None, :]).astype(bf16)     # (B, N)

    w1e = np.zeros((P, 2 * H), bf16)
    w1e[0:64] = (W1[1:] / 1024.0).astype(bf16)
    w1e[64] = (W1[0] / 1024.0).astype(bf16)
    w1e[65] = b1.astype(bf16)
    w1o = np.zeros((P, 2 * H), bf16)
    w1o[62] = (W1[0] / 1024.0).astype(bf16); w1o[63] = b1.astype(bf16)
    w1o[64:128] = (W1[1:] / 1024.0).astype(bf16)
    w2e = np.zeros((P, H), bf16)
    w2e[0:64] = (W2[1:] / 1024.0).astype(bf16)
    w2e[64] = (W2[0] / 1024.0).astype(bf16)
    w2e[65] = b2.astype(bf16)
    w2o = np.zeros((P, H), bf16)
    w2o[62] = (W2[0] / 1024.0).astype(bf16); w2o[63] = b2.astype(bf16)
    w2o[64:128] = (W2[1:] / 1024.0).astype(bf16)

    dh = (32.0 * d[None, :, None] * hidden).astype(f32)             # (B, N, H)

    in_maps = []
    for c in range(NCORES):
        bs = slice(BL * c, BL * c + BL)
        x1n = dh[bs][:, pi, :].transpose(1, 0, 2).reshape(N, BL * H)
        x1 = np.ascontiguousarray(
            x1n.reshape(KT, P, BL * H).transpose(1, 0, 2).reshape(P, KT * BL * H)
        ).astype(fp8)
        hrm = np.ascontiguousarray(
            hidden[bs].reshape(BL, NT, P, 2 * H).transpose(2, 0, 1, 3)
        ).astype(bf16)
        axc = axin_s[bs]                                            # (8, N) bf16
        axe = np.zeros((64, BL // 2, N), bf16)
        axe[0] = axc[0:BL:2]; axe[1] = 1.0
        axo = np.zeros((64, BL // 2, N), bf16)
        axo[62] = axc[1:BL:2]; axo[63] = 1.0
        in_maps.append({
            "a": a_pre, "x1": x1, "hrm": hrm,
            "axe": axe, "axo": axo,
            "w1e": w1e, "w1o": w1o, "w2e": w2e, "w2o": w2o,
        })
    return in_maps


LAST_RESULTS = None


def kernel(input_tensor, hidden, adj, W1, b1, W2, b2):
    global LAST_RESULTS
    if "nc" not in _CACHE:
        _CACHE["nc"] = _build()
    nc = _CACHE["nc"]
    in_maps = _prep_inputs(input_tensor, hidden, adj, W1, b1, W2, b2)
    res = run_bass_kernel_spmd(nc, in_maps, core_ids=list(range(NCORES)))
    LAST_RESULTS = res
    outs = [np.asarray(r["out"]).astype(np.float32) for r in res.results]
    return np.concatenate(outs, axis=0).reshape(B, N, H)


if __name__ == "__main__":
    rng = np.random.default_rng(0)
    inputs = {
        "input_tensor": rng.standard_normal((B, N), dtype=np.float32),
        "hidden": rng.standard_normal((B, N, H), dtype=np.float32),
        "adj": rng.random((N, N), dtype=np.float32),
        "W1": rng.standard_normal((H + 1, 2 * H), dtype=np.float32) * 0.15,
        "b1": np.full((2 * H,), 0.4, np.float32),
        "W2": rng.standard_normal((H + 1, H), dtype=np.float32) * 0.15,
        "b2": np.full((H,), 0.6, np.float32),
    }
    out = kernel(**inputs)
    print(out.shape, out.dtype)
